# revision 15
# baseline (speedup 1.0000x reference)
"""Trainium2 Bass kernel for nn_CodeLACE (sparse-attention transformer, 2 layers).

Distribution (8 NeuronCores): batch-split into two independent 4-core groups
(batch 0 -> cores 0-3, batch 1 -> cores 4-7). Within a group, core r (=c%4):
  - heads 4r..4r+3 of the attention (column shard of Wq/Wk/Wv),
  - expert r of the MoE (full I, all 1024 batch tokens),
  - token quarter r for the embedding+LN0 entry.
Activations live in T-layout (features on partitions, tokens on free dim),
replicated within the group. Per layer only two collectives (within groups):
AllGather of ctx head-slices, AllReduce (2 token-chunks) of gate-scaled
expert outputs.

Math tricks (validated against the jax reference in golden.py):
  - multiplicative-mask softmax: t = m*(exp(s/8)-1) + 1; ctx_u = t @ [v|1] is
    computed as (m*(e-1)) @ [v|1] + colsum([v|1]) (rank-1 fix via ACT bias);
    row 64 of ctx_u is the softmax denominator.
  - gates folded into the MoE epilogue: moe = sum_e g_e * expert_e(x).
  - LN over features in T-layout: stats via ones-matmul column sums,
    normalization with gpsimd partition-broadcast tiles.
"""
import numpy as np

import concourse.bass as bass
import concourse.mybir as mybir
import concourse.tile as tile
from concourse import bacc
from concourse.bass_utils import run_bass_kernel_spmd
from concourse.masks import make_identity

DT = mybir.dt
F32, BF16, I16 = DT.float32, DT.bfloat16, DT.int16
AF = mybir.ActivationFunctionType
ALU = mybir.AluOpType
AX = mybir.AxisListType

V, H, NH, HD = 32000, 1024, 16, 64
II, E, L = 4096, 4, 2
B, S = 2, 1024
EPS = 1e-12
GROUPS = [[0, 1, 2, 3], [4, 5, 6, 7]]
NCORES = 8

_NPBF16 = DT.np(BF16)

_CACHE = {}


def _sparse_masks():
    """Layer masks, exactly as the reference computes them (same jax config)."""
    if "masks" in _CACHE:
        return _CACHE["masks"]
    import jax
    import jax.numpy as jnp

    ms = []
    mkey = jax.random.key(42)
    for l in range(L):
        key = jax.random.fold_in(mkey, l)
        num_random = max(1, int(S * 0.1))
        i = jnp.arange(S)
        local = (jnp.abs(i[:, None] - i[None, :]) <= 2).astype(jnp.float32)
        r = jax.random.uniform(key, (S, S))
        idx = jax.lax.top_k(r, num_random)[1]
        rand = jnp.zeros((S, S), jnp.float32).at[i[:, None], idx].set(1.0)
        ms.append(np.asarray(jnp.maximum(local, rand), dtype=np.float32))
    _CACHE["masks"] = ms
    return ms


# --------------------------------------------------------------------------
# graph builder
# --------------------------------------------------------------------------

def build_graph(flags):
    """flags: dict of booleans for optional (zero in practice) bias/affine paths."""
    nc = bacc.Bacc("TRN2", target_bir_lowering=False, debug=False,
                   num_devices=NCORES)

    def inp(name, shape, dt):
        return nc.dram_tensor(name, list(shape), dt, kind="ExternalInput")

    tok_ext = inp("tok", (V, H), BF16)
    ids_ext = inp("ids", (128, 16), I16)
    pos_ext = inp("pos", (128, 2, H), F32)
    maskT_ext = inp("maskT", (L, 8, 128, S), BF16)
    wq_ext = inp("wq", (L, 8, 128, 256), BF16)
    wk_ext = inp("wk", (L, 8, 128, 256), BF16)
    wv_ext = inp("wv", (L, 8, 128, 256), BF16)
    ew1_ext = inp("ew1", (L, 32, 8, 128, 128), BF16)
    ew2_ext = inp("ew2", (L, 8, 32, 128, 128), BF16)
    gw_ext = inp("gw", (L, 8, 128, 4), BF16)
    oh_ext = inp("onehot", (4, 1), BF16)
    w45_ext = inp("w45", (8, 128, 45), BF16)
    ln0w_ext = inp("ln0w", (128, 8), F32)
    ln0b_ext = inp("ln0b", (128, 8), F32)
    if flags["lnw"]:
        ln1w_ext = inp("ln1w", (L, 128, 8), F32)
        ln1b_ext = inp("ln1b", (L, 128, 8), F32)
        ln2w_ext = inp("ln2w", (L, 128, 8), F32)
        ln2b_ext = inp("ln2b", (L, 128, 8), F32)
    if flags["qkvb"]:
        bq_ext = inp("bq", (L, 128, 2), F32)
        bk_ext = inp("bk", (L, 128, 2), F32)
        bv_ext = inp("bv", (L, 128, 2), F32)
    if flags["eb1"]:
        eb1_ext = inp("eb1", (L, 128, 32), F32)
    if flags["eb2"]:
        eb2_ext = inp("eb2", (L, 1, H), BF16)
    if flags["gb"]:
        gb_ext = inp("gateb", (L, 4, 1), F32)
    if flags["b45"]:
        b45_ext = inp("b45", (45, 1), F32)
    out_ext = nc.dram_tensor("out", [45], F32, kind="ExternalOutput")

    with tile.TileContext(nc) as tc, \
            tc.tile_pool(name="sb", bufs=1) as sb, \
            tc.tile_pool(name="ps", bufs=1, space="PSUM") as ps, \
            tc.tile_pool(name="dr", bufs=1, space="DRAM") as dr:
        # ---- constants ----
        ident = sb.tile([128, 128], BF16, tag="ident")
        make_identity(nc, ident)
        ones_bf = sb.tile([128, 1], BF16, tag="ones_bf")
        nc.gpsimd.memset(ones_bf[:], 1.0)
        oh_sb = sb.tile([4, 1], BF16, tag="oh")
        nc.sync.dma_start(out=oh_sb[:], in_=oh_ext.ap())
        w45_sb = sb.tile([128, 8, 45], BF16, tag="w45")
        for ht in range(8):
            nc.sync.dma_start(out=w45_sb[:, ht, :], in_=w45_ext.ap()[ht])
        ln0w_sb = sb.tile([128, 8], F32, tag="ln0w")
        ln0b_sb = sb.tile([128, 8], F32, tag="ln0b")
        nc.sync.dma_start(out=ln0w_sb[:], in_=ln0w_ext.ap())
        nc.sync.dma_start(out=ln0b_sb[:], in_=ln0b_ext.ap())
        if flags["b45"]:
            b45_sb = sb.tile([45, 1], F32, tag="b45")
            nc.sync.dma_start(out=b45_sb[:], in_=b45_ext.ap())

        # ---- embedding gather + LN0 (x-layout, per-quarter) ----
        ids_sb = sb.tile([128, 16], I16, tag="ids")
        nc.sync.dma_start(out=ids_sb[:], in_=ids_ext.ap())
        # gat shares the (later-used) z slot; pos shares the h1buf slot.
        gat = sb.tile([128, 2, H], BF16, tag="z", name="gat")
        nc.gpsimd.dma_gather(
            out_ap=gat[:], in_ap=tok_ext.ap(), idxs_ap=ids_sb[:],
            num_idxs=256, num_idxs_reg=256, elem_size=H,
        )
        pos_sb = sb.tile([128, 2, H], F32, tag="h1buf", name="pos_sb")
        nc.sync.dma_start(out=pos_sb[:], in_=pos_ext.ap())
        nc.vector.tensor_add(pos_sb[:], gat[:], pos_sb[:])  # x_emb, in place

        ag0_in = dr.tile([8, 128, 256], BF16, tag="ag0i")
        ag0_out = dr.tile([4, 8, 128, 256], BF16, tag="ag0o")

        for g in range(2):
            xg = pos_sb[:, g, :]
            s_ = sb.tile([128, 1], F32, tag="ln0r", bufs=9)
            nc.vector.tensor_reduce(s_[:], xg, axis=AX.X, op=ALU.add)
            sq = sb.tile([128, H], BF16, tag="zsq", bufs=1, name="sq0")
            nc.scalar.square(sq[:], xg)
            s2 = sb.tile([128, 1], F32, tag="ln0r", bufs=9)
            nc.vector.tensor_reduce(s2[:], sq[:], axis=AX.X, op=ALU.add)
            mu = sb.tile([128, 1], F32, tag="ln0r", bufs=9)
            nc.vector.tensor_scalar_mul(mu[:], s_[:], 1.0 / H)
            msq = sb.tile([128, 1], F32, tag="ln0r", bufs=9)
            nc.vector.tensor_scalar_mul(msq[:], s2[:], 1.0 / H)
            mu2 = sb.tile([128, 1], F32, tag="ln0r", bufs=9)
            nc.vector.tensor_mul(mu2[:], mu[:], mu[:])
            var = sb.tile([128, 1], F32, tag="ln0r", bufs=9)
            nc.vector.tensor_sub(var[:], msq[:], mu2[:])
            vpe = sb.tile([128, 1], F32, tag="ln0r", bufs=9)
            nc.vector.tensor_scalar_add(vpe[:], var[:], EPS)
            rcp = sb.tile([128, 1], F32, tag="ln0r", bufs=9)
            nc.vector.reciprocal(rcp[:], vpe[:])
            rs = sb.tile([128, 1], F32, tag="ln0r", bufs=9)
            nc.scalar.sqrt(rs[:], rcp[:])
            xn = sb.tile([128, H], BF16, tag="ln_t1", bufs=2, name="xn0")
            nc.vector.tensor_scalar(xn[:], xg, mu[:], rs[:],
                                    op0=ALU.subtract, op1=ALU.mult)
            for ht in range(8):
                tp = ps.tile([128, 128], BF16, tag="psA", bufs=4)
                nc.tensor.transpose(tp[:], xn[:, ht * 128:(ht + 1) * 128], ident[:])
                x0q = sb.tile([128, 128], BF16, tag="x0q", bufs=3)
                nc.vector.tensor_scalar(x0q[:], tp[:],
                                        ln0w_sb[:, ht:ht + 1], ln0b_sb[:, ht:ht + 1],
                                        op0=ALU.mult, op1=ALU.add)
                nc.sync.dma_start(out=ag0_in[ht, :, g * 128:(g + 1) * 128],
                                  in_=x0q[:])

        nc.gpsimd.collective_compute(
            "AllGather", ALU.bypass, replica_groups=GROUPS,
            ins=[ag0_in.opt()], outs=[ag0_out.opt()],
        )

        xT = sb.tile([128, 8, S], BF16, tag="xT", name="xT0")
        for ht in range(8):
            for c4 in range(4):
                nc.sync.dma_start(
                    out=xT[:, ht, c4 * 256:(c4 + 1) * 256],
                    in_=ag0_out[c4, ht],
                )

        # ================= layer loop =================
        for l in range(L):
            wq_sb = sb.tile([128, 8, 256], BF16, tag="wq")
            wk_sb = sb.tile([128, 8, 256], BF16, tag="wk")
            wv_sb = sb.tile([128, 8, 256], BF16, tag="wv")
            for kt in range(8):
                nc.sync.dma_start(out=wq_sb[:, kt, :], in_=wq_ext.ap()[l, kt])
                nc.sync.dma_start(out=wk_sb[:, kt, :], in_=wk_ext.ap()[l, kt])
                nc.sync.dma_start(out=wv_sb[:, kt, :], in_=wv_ext.ap()[l, kt])
            gw_sb = sb.tile([128, 8, 4], BF16, tag="gw")
            for kt in range(8):
                nc.sync.dma_start(out=gw_sb[:, kt, :], in_=gw_ext.ap()[l, kt])
            if flags["qkvb"]:
                bq_sb = sb.tile([128, 2], F32, tag="bq")
                bk_sb = sb.tile([128, 2], F32, tag="bk")
                bv_sb = sb.tile([128, 2], F32, tag="bv")
                nc.sync.dma_start(out=bq_sb[:], in_=bq_ext.ap()[l])
                nc.sync.dma_start(out=bk_sb[:], in_=bk_ext.ap()[l])
                nc.sync.dma_start(out=bv_sb[:], in_=bv_ext.ap()[l])
            if flags["lnw"]:
                ln1w_sb = sb.tile([128, 8], F32, tag="ln1w")
                ln1b_sb = sb.tile([128, 8], F32, tag="ln1b")
                ln2w_sb = sb.tile([128, 8], F32, tag="ln2w")
                ln2b_sb = sb.tile([128, 8], F32, tag="ln2b")
                nc.sync.dma_start(out=ln1w_sb[:], in_=ln1w_ext.ap()[l])
                nc.sync.dma_start(out=ln1b_sb[:], in_=ln1b_ext.ap()[l])
                nc.sync.dma_start(out=ln2w_sb[:], in_=ln2w_ext.ap()[l])
                nc.sync.dma_start(out=ln2b_sb[:], in_=ln2b_ext.ap()[l])
            if flags["eb1"]:
                eb1_sb = sb.tile([128, 32], F32, tag="eb1")
                nc.sync.dma_start(out=eb1_sb[:], in_=eb1_ext.ap()[l])
            if flags["eb2"]:
                eb2_sb = sb.tile([1, H], BF16, tag="eb2")
                nc.sync.dma_start(out=eb2_sb[:], in_=eb2_ext.ap()[l])
            if flags["gb"]:
                gb_sb = sb.tile([4, 1], F32, tag="gateb")
                nc.sync.dma_start(out=gb_sb[:], in_=gb_ext.ap()[l])

            # --- QKV projections (local 4 heads = 256 cols) ---
            qT = sb.tile([128, 2, S], BF16, tag="qT")
            kT = sb.tile([128, 2, S], BF16, tag="kT")
            vT = sb.tile([128, 2, S], BF16, tag="vT")
            for pi, (wsb, dst) in enumerate(((wq_sb, qT), (wk_sb, kT),
                                            (wv_sb, vT))):
                for mt in range(2):
                    pa = ps.tile([128, 512], F32, tag="psA", bufs=4, name="pa")
                    pb = ps.tile([128, 512], F32, tag="psA", bufs=4, name="pb")
                    for kt in range(8):
                        w_ap = wsb[:, kt, mt * 128:(mt + 1) * 128]
                        nc.tensor.matmul(pa[:], lhsT=w_ap, rhs=xT[:, kt, 0:512],
                                         start=(kt == 0), stop=(kt == 7))
                        nc.tensor.matmul(pb[:], lhsT=w_ap, rhs=xT[:, kt, 512:1024],
                                         start=(kt == 0), stop=(kt == 7))
                    if flags["qkvb"]:
                        bt = (bq_sb, bk_sb, bv_sb)[pi]
                        nc.scalar.activation(dst[:, mt, 0:512], pa[:], AF.Identity,
                                             bias=bt[:, mt:mt + 1])
                        nc.scalar.activation(dst[:, mt, 512:1024], pb[:], AF.Identity,
                                             bias=bt[:, mt:mt + 1])
                    else:
                        nc.scalar.copy(dst[:, mt, 0:512], pa[:])
                        nc.scalar.copy(dst[:, mt, 512:1024], pb[:])

            # --- v_aug: [128 kpos, head, ktile, 65] ; col 64 = ones ---
            vaug = sb.tile([128, 4, 8, 65], BF16, tag="vaug")
            nc.gpsimd.memset(vaug[:, :, :, 64:65], 1.0)
            for j in range(4):
                off, pt = 64 * (j % 2), j // 2
                idn = ident[off:off + 64, off:off + 64]
                for kt in range(8):
                    tp = ps.tile([128, 64], BF16, tag="psA", bufs=4)
                    nc.tensor.transpose(
                        tp[:], vT[off:off + 64, pt, kt * 128:(kt + 1) * 128], idn)
                    nc.scalar.copy(vaug[:, j, kt, 0:64], tp[:])

            # --- attention, two heads at a time (streamed mask tiles) ---
            ctxn = sb.tile([64, 4, S], BF16, tag="ctxn")
            for hp in range(2):
                pcs = {}
                for j in (2 * hp, 2 * hp + 1):
                    cs_ps = ps.tile([65, 1], F32, tag="psC", bufs=4)
                    for kt in range(8):
                        nc.tensor.matmul(cs_ps[:], lhsT=vaug[:, j, kt, :],
                                         rhs=ones_bf[:],
                                         start=(kt == 0), stop=(kt == 7))
                    cs_sb = sb.tile([65, 1], F32, tag="cs", bufs=2)
                    nc.scalar.copy(cs_sb[:], cs_ps[:])
                    pcs[j] = (cs_sb,
                              [ps.tile([65, 512], F32, tag="psC", bufs=4,
                                       name=f"pc{j}_{c_}") for c_ in range(2)])
                for kt in range(8):
                    mtile = sb.tile([128, S], BF16, tag="mTt", bufs=2, name="mtile")
                    nc.sync.dma_start(out=mtile[:], in_=maskT_ext.ap()[l, kt])
                    for j in (2 * hp, 2 * hp + 1):
                        off, pt = 64 * (j % 2), j // 2
                        for ch in range(2):
                            s_ps = ps.tile([128, 512], F32, tag="psA", bufs=4)
                            nc.tensor.matmul(
                                s_ps[:],
                                lhsT=kT[off:off + 64, pt, kt * 128:(kt + 1) * 128],
                                rhs=qT[off:off + 64, pt, ch * 512:(ch + 1) * 512],
                                start=True, stop=True)
                            e1 = sb.tile([128, 512], BF16, tag="e1", bufs=2)
                            nc.scalar.activation(e1[:], s_ps[:], AF.Exp, scale=0.125)
                            tpr = sb.tile([128, 512], BF16, tag="tpr", bufs=2)
                            nc.vector.scalar_tensor_tensor(
                                tpr[:], in0=e1[:], scalar=1.0,
                                in1=mtile[:, ch * 512:(ch + 1) * 512],
                                op0=ALU.subtract, op1=ALU.mult)
                            nc.tensor.matmul(pcs[j][1][ch][:],
                                             lhsT=vaug[:, j, kt, :], rhs=tpr[:],
                                             start=(kt == 0), stop=(kt == 7))
                for j in (2 * hp, 2 * hp + 1):
                    cs_sb, pc = pcs[j]
                    ctxu = sb.tile([65, S], BF16, tag="ctxu", bufs=2)
                    for ch in range(2):
                        nc.scalar.activation(ctxu[:, ch * 512:(ch + 1) * 512],
                                             pc[ch][:], AF.Identity, bias=cs_sb[:])
                    rsum = sb.tile([1, S], BF16, tag="rsum", bufs=2)
                    nc.sync.dma_start(out=rsum[:], in_=ctxu[64:65, :])
                    rec = sb.tile([1, S], F32, tag="rec", bufs=1)
                    nc.vector.reciprocal(rec[:], rsum[:])
                    rec16 = sb.tile([1, S], BF16, tag="rec16", bufs=2)
                    nc.scalar.copy(rec16[:], rec[:])
                    recb = sb.tile([64, S], BF16, tag="recb", bufs=2)
                    nc.gpsimd.partition_broadcast(recb[:], rec16[:], channels=64)
                    nc.vector.tensor_mul(ctxn[:, j, :], ctxu[0:64, :], recb[:])

            # --- AllGather ctx head-slices; residual z1 = xT + ctx ---
            agc_in = dr.tile([2, 128, S], BF16, tag="agci")
            agc_out = dr.tile([4, 2, 128, S], BF16, tag="agco")
            for j in range(4):
                off, pt = 64 * (j % 2), j // 2
                nc.sync.dma_start(out=agc_in[pt, off:off + 64, :], in_=ctxn[:, j, :])
            nc.gpsimd.collective_compute(
                "AllGather", ALU.bypass, replica_groups=GROUPS,
                ins=[agc_in.opt()], outs=[agc_out.opt()],
            )
            z1 = sb.tile([128, 8, S], BF16, tag="z", name="z1")
            for ht in range(8):
                agt = sb.tile([128, S], BF16, tag="agt", bufs=2, name="agt1")
                nc.sync.dma_start(out=agt[:], in_=agc_out[ht // 2, ht % 2])
                nc.vector.tensor_add(z1[:, ht, :], xT[:, ht, :], agt[:])

            x1T = sb.tile([128, 8, S], BF16, tag="x1T")
            _emit_lnT(nc, sb, ps, z1, x1T, ones_bf,
                      (ln1w_sb, ln1b_sb) if flags["lnw"] else None)

            # --- gates (replicated): g_e row + partition broadcast ---
            eg = sb.tile([4, S], BF16, tag="eg")
            for ch in range(2):
                sl = slice(ch * 512, (ch + 1) * 512)
                glps = ps.tile([4, 512], F32, tag="psC", bufs=4, name="glps")
                for kt in range(8):
                    nc.tensor.matmul(glps[:], lhsT=gw_sb[:, kt, :],
                                     rhs=x1T[:, kt, sl],
                                     start=(kt == 0), stop=(kt == 7))
                if flags["gb"]:
                    nc.scalar.activation(eg[:, sl], glps[:], AF.Exp, bias=gb_sb[:])
                else:
                    nc.scalar.activation(eg[:, sl], glps[:], AF.Exp)
            grec = sb.tile([1, S], F32, tag="grec")
            ge = sb.tile([1, S], BF16, tag="ge")
            for ch in range(2):
                sl = slice(ch * 512, (ch + 1) * 512)
                gs_ps = ps.tile([1, 512], F32, tag="psA", bufs=4, name="gs_ps")
                ge_ps = ps.tile([1, 512], F32, tag="psA", bufs=4, name="ge_ps")
                nc.tensor.matmul(gs_ps[:], lhsT=ones_bf[0:4, :], rhs=eg[:, sl],
                                 start=True, stop=True)
                nc.tensor.matmul(ge_ps[:], lhsT=oh_sb[:], rhs=eg[:, sl],
                                 start=True, stop=True)
                nc.vector.reciprocal(grec[:, sl], gs_ps[:])
                nc.vector.tensor_mul(ge[:, sl], ge_ps[:], grec[:, sl])
            gbc = sb.tile([128, S], BF16, tag="gbc")
            nc.gpsimd.partition_broadcast(gbc[:], ge[:], channels=128)

            # --- MoE expert GEMMs, gate-scaled epilogue, chunked AllReduce ---
            ar_in = [dr.tile([8, 128, 512], BF16, tag=f"ari{c_}", name=f"ari{c_}")
                     for c_ in range(2)]
            ar_out = [dr.tile([8, 128, 512], BF16, tag=f"aro{c_}", name=f"aro{c_}")
                      for c_ in range(2)]
            h1buf = sb.tile([128, 32, 512], BF16, tag="h1buf", name="h1buf")
            for ch in range(2):
                sl = slice(ch * 512, (ch + 1) * 512)
                for it in range(32):
                    p1 = ps.tile([128, 512], F32, tag="psA", bufs=4)
                    for kt in range(8):
                        w1t = sb.tile([128, 128], BF16, tag="ew1t", bufs=16)
                        nc.sync.dma_start(out=w1t[:], in_=ew1_ext.ap()[l, it, kt])
                        nc.tensor.matmul(p1[:], lhsT=w1t[:], rhs=x1T[:, kt, sl],
                                         start=(kt == 0), stop=(kt == 7))
                    if flags["eb1"]:
                        nc.scalar.activation(h1buf[:, it, :], p1[:], AF.Relu,
                                             bias=eb1_sb[:, it:it + 1])
                    else:
                        nc.scalar.activation(h1buf[:, it, :], p1[:], AF.Relu)
                for ht in range(8):
                    p2 = ps.tile([128, 512], F32, tag="psA", bufs=4)
                    for it in range(32):
                        w2t = sb.tile([128, 128], BF16, tag="ew2t", bufs=16)
                        nc.sync.dma_start(out=w2t[:], in_=ew2_ext.ap()[l, ht, it])
                        nc.tensor.matmul(p2[:], lhsT=w2t[:], rhs=h1buf[:, it, :],
                                         start=(it == 0), stop=(it == 31))
                    if flags["eb2"]:
                        ones_row = sb.tile([1, 512], BF16, tag="ones_row")
                        nc.gpsimd.memset(ones_row[:], 1.0)
                        nc.tensor.matmul(
                            p2[:], lhsT=eb2_sb[:, ht * 128:(ht + 1) * 128],
                            rhs=ones_row[:], start=False, stop=True,
                            skip_group_check=True)
                    mo = sb.tile([128, 512], BF16, tag="mo", bufs=3)
                    nc.vector.tensor_mul(mo[:], p2[:], gbc[:, sl])
                    nc.sync.dma_start(out=ar_in[ch][ht], in_=mo[:])
                nc.gpsimd.collective_compute(
                    "AllReduce", ALU.add, replica_groups=GROUPS,
                    ins=[ar_in[ch].opt()], outs=[ar_out[ch].opt()],
                )

            # --- residual z2 = x1T + moe; LN2 -> next xT ---
            z2 = sb.tile([128, 8, S], BF16, tag="z", name="z2")
            for ch in range(2):
                sl = slice(ch * 512, (ch + 1) * 512)
                for ht in range(8):
                    art = sb.tile([128, 512], BF16, tag="agt", bufs=2, name="agt2")
                    nc.sync.dma_start(out=art[:], in_=ar_out[ch][ht])
                    nc.vector.tensor_add(z2[:, ht, sl], x1T[:, ht, sl], art[:])
            xT = sb.tile([128, 8, S], BF16, tag="xT", name=f"xT{l + 1}")
            _emit_lnT(nc, sb, ps, z2, xT, ones_bf,
                      (ln2w_sb, ln2b_sb) if flags["lnw"] else None)

        # ---- pooled mean + output heads ----
        poolbf = sb.tile([128, 8], BF16, tag="poolbf")
        for ht in range(8):
            pr = sb.tile([128, 1], F32, tag="poolr", bufs=2)
            nc.vector.tensor_reduce(pr[:], xT[:, ht, :], axis=AX.X, op=ALU.add)
            nc.scalar.copy(poolbf[:, ht:ht + 1], pr[:])
        fp = ps.tile([45, 1], F32, tag="psC", bufs=4)
        for ht in range(8):
            nc.tensor.matmul(fp[:], lhsT=w45_sb[:, ht, :],
                             rhs=poolbf[:, ht:ht + 1],
                             start=(ht == 0), stop=(ht == 7))
        o_sb = sb.tile([45, 1], F32, tag="o45")
        if flags["b45"]:
            nc.scalar.activation(o_sb[:], fp[:], AF.Identity, bias=b45_sb[:])
        else:
            nc.scalar.copy(o_sb[:], fp[:])
        nc.sync.dma_start(out=out_ext.ap(), in_=o_sb[:])

    nc.compile()
    return nc


def _emit_lnT(nc, sb, ps, z, out, ones_bf, affine):
    """LayerNorm over the feature (partition) axis of z [128, 8, S] -> out."""
    sum_ps = [ps.tile([1, 512], F32, tag="psC", bufs=4, name=f"lns{c_}")
              for c_ in range(2)]
    sq_ps = [ps.tile([1, 512], F32, tag="psC", bufs=4, name=f"lnq{c_}")
             for c_ in range(2)]
    for ht in range(8):
        zsq = sb.tile([128, S], BF16, tag="zsq", bufs=1)
        nc.scalar.square(zsq[:], z[:, ht, :])
        for ch in range(2):
            sl = slice(ch * 512, (ch + 1) * 512)
            nc.tensor.matmul(sum_ps[ch][:], lhsT=ones_bf[:], rhs=z[:, ht, sl],
                             start=(ht == 0), stop=(ht == 7))
            nc.tensor.matmul(sq_ps[ch][:], lhsT=ones_bf[:], rhs=zsq[:, sl],
                             start=(ht == 0), stop=(ht == 7))
    mu = sb.tile([1, S], F32, tag="ln_mu", bufs=1)
    msq = sb.tile([1, S], F32, tag="ln_r", bufs=3)
    for ch in range(2):
        sl = slice(ch * 512, (ch + 1) * 512)
        nc.vector.tensor_scalar_mul(mu[:, sl], sum_ps[ch][:], 1.0 / H)
        nc.vector.tensor_scalar_mul(msq[:, sl], sq_ps[ch][:], 1.0 / H)
    mu2 = sb.tile([1, S], F32, tag="ln_r", bufs=3)
    nc.vector.tensor_mul(mu2[:], mu[:], mu[:])
    var = sb.tile([1, S], F32, tag="ln_r", bufs=3)
    nc.vector.tensor_sub(var[:], msq[:], mu2[:])
    vpe = sb.tile([1, S], F32, tag="ln_r", bufs=3)
    nc.vector.tensor_scalar_add(vpe[:], var[:], EPS)
    rcp = sb.tile([1, S], F32, tag="ln_r", bufs=3)
    nc.vector.reciprocal(rcp[:], vpe[:])
    rs = sb.tile([1, S], F32, tag="ln_rs", bufs=1)
    nc.scalar.sqrt(rs[:], rcp[:])
    mub16 = sb.tile([1, S], BF16, tag="ln_mu16", bufs=1)
    nc.scalar.copy(mub16[:], mu[:])
    rsb16 = sb.tile([1, S], BF16, tag="ln_rs16", bufs=1)
    nc.scalar.copy(rsb16[:], rs[:])
    mub = sb.tile([128, S], BF16, tag="ln_mub", bufs=1)
    nc.gpsimd.partition_broadcast(mub[:], mub16[:], channels=128)
    rsb = sb.tile([128, S], BF16, tag="ln_rsb", bufs=1)
    nc.gpsimd.partition_broadcast(rsb[:], rsb16[:], channels=128)
    for ht in range(8):
        t1 = sb.tile([128, S], BF16, tag="ln_t1", bufs=2)
        nc.vector.tensor_sub(t1[:], z[:, ht, :], mub[:])
        if affine is None:
            nc.vector.tensor_mul(out[:, ht, :], t1[:], rsb[:])
        else:
            w_sb, b_sb = affine
            t2 = sb.tile([128, S], BF16, tag="ln_t1", bufs=2, name="t2")
            nc.vector.tensor_mul(t2[:], t1[:], rsb[:])
            nc.vector.tensor_scalar(out[:, ht, :], t2[:],
                                    w_sb[:, ht:ht + 1], b_sb[:, ht:ht + 1],
                                    op0=ALU.mult, op1=ALU.add)


# --------------------------------------------------------------------------
# host marshaling
# --------------------------------------------------------------------------

def _to_bf16(x):
    return np.asarray(x, dtype=np.float32).astype(_NPBF16)


def make_in_maps(inputs, masks):
    """Build the 8 per-core input dicts from the full-model inputs."""
    f = {k: np.asarray(v) for k, v in inputs.items()}
    flags = {
        "qkvb": bool(np.any(f["bq"]) or np.any(f["bk"]) or np.any(f["bv"])),
        "lnw": bool(np.any(f["ln1_w"] != 1) or np.any(f["ln1_b"]) or
                    np.any(f["ln2_w"] != 1) or np.any(f["ln2_b"])),
        "eb1": bool(np.any(f["eb1"])),
        "eb2": bool(np.any(f["eb2"])),
        "gb": bool(np.any(f["gate_b"])),
        "b45": bool(np.any(f["syn_b"]) or np.any(f["sem_b"]) or
                    np.any(f["prag_b"])),
    }

    tok = _to_bf16(f["tok_emb"])
    maskT = np.stack([m.T.reshape(8, 128, S) for m in masks]).astype(_NPBF16)
    W45 = np.concatenate([f["syn_w"], f["sem_w"], f["prag_w"]], axis=1)
    w45 = (W45.astype(np.float32) / float(S)).reshape(8, 128, 45).astype(_NPBF16)
    ln0w = f["ln0_w"].astype(np.float32).reshape(8, 128).T.copy()
    ln0b = f["ln0_b"].astype(np.float32).reshape(8, 128).T.copy()
    ew1 = np.stack([
        np.stack([f["ew1"][l, e].astype(np.float32)
                  .reshape(8, 128, 32, 128).transpose(2, 0, 1, 3)
                  for e in range(E)])
        for l in range(L)
    ]).astype(_NPBF16)  # [L, E, 32, 8, 128, 128]
    ew2 = np.stack([
        np.stack([f["ew2"][l, e].astype(np.float32)
                  .reshape(32, 128, 8, 128).transpose(2, 0, 1, 3)
                  for e in range(E)])
        for l in range(L)
    ]).astype(_NPBF16)  # [L, E, 8, 32, 128, 128]
    gw = np.stack([f["gate_w"][l].astype(np.float32).reshape(8, 128, 4)
                   for l in range(L)]).astype(_NPBF16)

    in_maps = []
    for c in range(NCORES):
        b, r = c // 4, c % 4
        ids_q = np.asarray(f["input_ids"][b, r * 256:(r + 1) * 256]).astype(np.int64)
        ids = np.tile(ids_q.reshape(16, 16).T.astype(np.int16), (8, 1))
        pos = (f["pos_emb"][r * 256:(r + 1) * 256].astype(np.float32)
               .reshape(2, 128, H).transpose(1, 0, 2).copy())
        cols = slice(256 * r, 256 * (r + 1))
        m = {
            "tok": tok,
            "ids": ids,
            "pos": pos,
            "maskT": maskT,
            "wq": np.stack([f["Wq"][l][:, cols].astype(np.float32)
                            .reshape(8, 128, 256) for l in range(L)]).astype(_NPBF16),
            "wk": np.stack([f["Wk"][l][:, cols].astype(np.float32)
                            .reshape(8, 128, 256) for l in range(L)]).astype(_NPBF16),
            "wv": np.stack([f["Wv"][l][:, cols].astype(np.float32)
                            .reshape(8, 128, 256) for l in range(L)]).astype(_NPBF16),
            "ew1": ew1[:, r],
            "ew2": ew2[:, r],
            "gw": gw,
            "onehot": np.eye(4, dtype=np.float32)[:, r:r + 1].astype(_NPBF16),
            "w45": w45,
            "ln0w": ln0w,
            "ln0b": ln0b,
        }
        if flags["qkvb"]:
            for nm, src in (("bq", "bq"), ("bk", "bk"), ("bv", "bv")):
                m[nm] = np.stack([f[src][l, cols].astype(np.float32)
                                  .reshape(2, 128).T.copy() for l in range(L)])
        if flags["lnw"]:
            for nm, src in (("ln1w", "ln1_w"), ("ln1b", "ln1_b"),
                            ("ln2w", "ln2_w"), ("ln2b", "ln2_b")):
                m[nm] = np.stack([f[src][l].astype(np.float32)
                                  .reshape(8, 128).T.copy() for l in range(L)])
        if flags["eb1"]:
            m["eb1"] = np.stack([f["eb1"][l, r].astype(np.float32)
                                 .reshape(32, 128).T.copy() for l in range(L)])
        if flags["eb2"]:
            m["eb2"] = np.stack([f["eb2"][l, r].astype(np.float32)
                                 .reshape(1, H) for l in range(L)]).astype(_NPBF16)
        if flags["gb"]:
            m["gateb"] = f["gate_b"].astype(np.float32).reshape(L, 4, 1)
        if flags["b45"]:
            m["b45"] = np.concatenate(
                [f["syn_b"], f["sem_b"], f["prag_b"]]
            ).astype(np.float32).reshape(45, 1)
        in_maps.append(m)
    return in_maps, flags


def get_graph(flags):
    key = tuple(sorted(flags.items()))
    if key not in _CACHE:
        _CACHE[key] = build_graph(flags)
    return _CACHE[key]


def kernel(**inputs):
    masks = _sparse_masks()
    in_maps, flags = make_in_maps(inputs, masks)
    nc = get_graph(flags)
    res = run_bass_kernel_spmd(nc, in_maps, core_ids=list(range(NCORES)))
    o0 = np.asarray(res.results[0]["out"], np.float32)
    o1 = np.asarray(res.results[4]["out"], np.float32)
    outs = np.stack([o0, o1])  # [2, 45]
    return (outs[:, :10].copy(), outs[:, 10:30].copy(), outs[:, 30:45].copy())
